# revision 19
# baseline (speedup 1.0000x reference)
"""GNN message-passing kernel for Trainium2 (Bass/Tile), 8-core SPMD.

Model (from the reference):
  h0 = relu(x @ W_in.T + b_in).T            # [500, B] -> vertices 0..500
  for l in 1..7:   agg = segment_sum(w_edge * h[edge_src]) ; h_l = relu(agg)
  out = h[out_verts].T @ W_out.T + b_out    # [B, 10]

Device strategy:
  - Data-parallel over batch: 8 cores x 256 columns each.
  - The sparse per-layer aggregation is cast as a dense matmul
    agg = A_l @ h_lower, where A_l ([500 x l*500], 32 nnz/row) is built
    on the host from (edge_src, edge_dst_local, w_edge) and streamed
    from HBM in bf16.
  - Vertex space padded to 512/layer so every layer is exactly 4
    partition tiles of 128; all matmul tiling is then uniform.
  - out_verts handling: highway vertices (out_verts below the last
    layer) are forwarded into unused pad rows of layer 7 via 1.0
    pass-through entries in A_7 (relu-idempotent since h >= 0), and
    b_out rides a constant-1 pad vertex (seeded through b_in's pad
    bias).  The output head then contracts only the 4 last-layer
    tiles, batch-major with n=10 moving columns, and the result DMAs
    out directly as [batch, 10].

Schedule notes (tuned against the TimelineSim cost model):
  - Warmup dummy matmuls keep the PE busy from t~0 so the p-state ramp
    (0.65/1.2 GHz until 3us continuously busy) finishes before real
    matmuls start, and they plug DMA-arrival gaps in the input layer.
  - Input layer is kt-major so it consumes W_in k-slabs in DMA arrival
    order; A chunk 0 is queued right behind them.
  - Small constant DMAs (bias, W_out head) issue from the gpsimd queue
    (SWDGE) so they stay off the serialized HWDGE generator.
  - Layer 7 ends m-major so the four PSUM groups stop staggered, and
    their relus run on scalar/vector/gpsimd in parallel to shorten the
    tail into the output head.
"""

import sys

try:
    import concourse  # noqa: F401  (provided by the axon site-path)
except ImportError:
    sys.path.insert(0, "/opt/trn_rl_repo")

import numpy as np
from ml_dtypes import bfloat16

# ---- problem geometry (fixed by the problem spec) ----
B = 2048            # total batch
NC = 8              # cores
BL = B // NC        # 256 batch columns per core
IN_DIM = 784
K_IN = 896          # 784 padded to 7*128
PER = 500           # vertices per layer
PAD = 512           # padded vertices per layer (4*128)
L = 8               # layers (layer 0 = input layer)
NT = 4 * L          # 32 h tiles of 128 vertices
OUT_DIM = 10
# A rows: layer l (1..7) contributes l*512 padded source rows
A_ROWS = PAD * (L * (L - 1) // 2)   # 14336
N_CHUNK = A_ROWS // PAD             # 28 chunks of 512 rows (4 k-tiles)
MAX_HW = 11          # highway vertices foldable into layer-7 pad rows

# schedule tuning knobs (dummy warmup matmuls, n=64 columns each)
N_DUMMY_HEAD = 4    # before the first input matmul
N_DUMMY_KT0 = 0     # between input kt0 and kt1 (W_in slab arrival gap)
N_DUMMY_KT3 = 0     # between input kt3 and kt4 (W_in slab arrival gap)
HEAD_LAG = 7         # hidden matmuls of block m+1 issued before head mm m

_PROG = None  # compiled program cache
_LAST_IN_MAPS = None  # kept for external profiling harnesses


def _build_program():
    from concourse import bacc, tile
    import concourse.mybir as mybir

    f32 = mybir.dt.float32
    bf16 = mybir.dt.bfloat16
    AF = mybir.ActivationFunctionType

    nc = bacc.Bacc(None, target_bir_lowering=False)
    xT_d = nc.dram_tensor("xT", [128, 7, BL], bf16, kind="ExternalInput")
    win_d = nc.dram_tensor("W_inT", [128, 7, 4, 128], bf16, kind="ExternalInput")
    bin_d = nc.dram_tensor("b_inP", [128, 4], f32, kind="ExternalInput")
    a_d = nc.dram_tensor("A", [N_CHUNK, 128, 4, PAD], bf16, kind="ExternalInput")
    wout_d = nc.dram_tensor("W_outP", [128, 4, OUT_DIM], bf16, kind="ExternalInput")
    out_d = nc.dram_tensor("out", [128, 2, OUT_DIM], f32, kind="ExternalOutput")

    with tile.TileContext(nc) as tc:
        with (
            tc.tile_pool(name="const", bufs=1) as cpool,
            tc.tile_pool(name="hbuf", bufs=1) as hpool,
            tc.tile_pool(name="astream", bufs=8) as apool,
            tc.tile_pool(name="ps", bufs=8, space="PSUM") as ppool,
            tc.tile_pool(name="outs", bufs=1) as spool,
        ):
            # ---- input DMAs: W_in kt0 and xT lead, A chunk 0 close behind
            win_s = cpool.tile([128, 7, 4, 128], bf16)
            xt_s = cpool.tile([128, 7, BL], bf16)
            nc.sync.dma_start(win_s[:, 0:1], win_d[:, 0:1])
            nc.sync.dma_start(xt_s[:, 0:2], xT_d[:, 0:2])
            nc.sync.dma_start(win_s[:, 1:4], win_d[:, 1:4])
            nc.sync.dma_start(xt_s[:, 2:7], xT_d[:, 2:7])
            nc.sync.dma_start(win_s[:, 4:7], win_d[:, 4:7])
            # ---- PE warmup: keep the engine busy from t~0 so the
            # p-state ramp completes before the first real matmul.
            scratch = cpool.tile([128, 64], bf16)
            nc.gpsimd.memset(scratch[:], 0.0)
            # small constants via SWDGE (gpsimd) to stay off HWDGE
            bin_s = cpool.tile([128, 4], f32)
            wout_s = cpool.tile([128, 4, OUT_DIM], bf16)
            nc.gpsimd.dma_start(bin_s[:], bin_d[:])
            nc.gpsimd.dma_start(wout_s[:], wout_d[:])
            psd = ppool.tile([64, 64], f32, tag="ps", name="psd")

            def dummy(n):
                for _ in range(n):
                    nc.tensor.matmul(
                        psd[:], scratch[:], scratch[:],
                        start=True, stop=True, skip_group_check=True,
                    )

            dummy(N_DUMMY_HEAD)

            h = hpool.tile([128, NT, BL], bf16)

            # ---- input layer (kt-major): h[0:4] = relu(W_in.T.T @ xT + b)
            psin = [ppool.tile([128, BL], f32, tag="ps", name=f"pi{m}")
                    for m in range(4)]
            for kt in range(7):
                for m in range(4):
                    nc.tensor.matmul(
                        psin[m][:],
                        win_s[:, kt, m, :],
                        xt_s[:, kt, :],
                        start=(kt == 0),
                        stop=(kt == 6),
                    )
                if kt == 0:
                    dummy(N_DUMMY_KT0)
                elif kt == 3:
                    dummy(N_DUMMY_KT3)
            for m in range(4):
                nc.scalar.activation(
                    h[:, m, :], psin[m][:], AF.Relu, bias=bin_s[:, m:m + 1]
                )

            # ---- hidden layers: h[4l..4l+4] = relu(A_l @ h[0:4l]) ----
            chunk = 0
            for l in range(1, L):
                nkt = 4 * l
                a_tiles = []
                for c in range(l):
                    at = apool.tile([128, 4, PAD], bf16, tag="achunk", name="at")
                    nc.sync.dma_start(at[:], a_d[chunk])
                    a_tiles.append(at)
                    chunk += 1
                pls = [
                    ppool.tile([128, BL], f32, tag="ps", name=f"pl{m}")
                    for m in range(4)
                ]
                if l < L - 1:
                    for kt in range(nkt):
                        a_s = a_tiles[kt // 4]
                        for m in range(4):
                            nc.tensor.matmul(
                                pls[m][:],
                                a_s[:, kt % 4, m * 128:(m + 1) * 128],
                                h[:, kt, :],
                                start=(kt == 0),
                                stop=(kt == nkt - 1),
                            )
                    for m in range(4):
                        nc.scalar.activation(
                            h[:, 4 * l + m, :], pls[m][:], AF.Relu, bias=0.0
                        )
                else:
                    # Last layer: kt-major until the last two chunks, then
                    # m-major so the four PSUM stops stagger ~850ns apart.
                    # Tile m's relu and output-head matmuls then hide under
                    # tile m+1's remaining contraction; only the final
                    # tile's relu + head matmuls sit on the tail.
                    pso = ppool.tile([128, 2, OUT_DIM], f32, tag="ps",
                                     name="pso")

                    def head_mms(t):
                        for half in range(2):
                            nc.tensor.matmul(
                                pso[:, half, :],
                                h[:, 28 + t, half * 128:(half + 1) * 128],
                                wout_s[:, t, :],
                                start=(t == 0),
                                stop=(t == 3),
                            )

                    kt_major = nkt - 8
                    for kt in range(kt_major):
                        a_s = a_tiles[kt // 4]
                        for m in range(4):
                            nc.tensor.matmul(
                                pls[m][:],
                                a_s[:, kt % 4, m * 128:(m + 1) * 128],
                                h[:, kt, :],
                                start=(kt == 0),
                                stop=False,
                            )
                    for m in range(3):
                        for kt in range(kt_major, nkt):
                            nc.tensor.matmul(
                                pls[m][:],
                                a_tiles[kt // 4][:, kt % 4,
                                                 m * 128:(m + 1) * 128],
                                h[:, kt, :],
                                start=False,
                                stop=(kt == nkt - 1),
                            )
                            if m > 0 and kt == kt_major + HEAD_LAG:
                                head_mms(m - 1)
                        nc.scalar.activation(
                            h[:, 4 * l + m, :], pls[m][:], AF.Relu, bias=0.0)
                    # final tile m=3: contract per batch-half so the two
                    # PSUM stops stagger; relu (DVE) and the head matmul
                    # for each half pipeline into the tail.
                    for hh in range(2):
                        bs = slice(hh * 128, (hh + 1) * 128)
                        for kt in range(kt_major, nkt):
                            if hh == 0 and kt == kt_major + HEAD_LAG:
                                head_mms(2)
                            nc.tensor.matmul(
                                pls[3][:, bs],
                                a_tiles[kt // 4][:, kt % 4, 384:512],
                                h[:, kt, bs],
                                start=False,
                                stop=(kt == nkt - 1),
                                skip_group_check=True,
                            )
                        nc.vector.tensor_scalar_max(
                            h[:, 4 * l + 3, bs], pls[3][:, bs], 0.0)
                    for hh in range(2):
                        nc.tensor.matmul(
                            pso[:, hh, :],
                            h[:, 31, hh * 128:(hh + 1) * 128],
                            wout_s[:, 3, :],
                            start=False,
                            stop=True,
                        )
            out_s = spool.tile([128, 2, OUT_DIM], f32)
            nc.vector.tensor_copy(out_s[:], pso[:])
            nc.gpsimd.dma_start(out_d[:], out_s[:])

    nc.compile()
    return nc


def _pack_ptiles(arr2d, n_tiles):
    """[n_tiles*128, F] row-major -> [128, n_tiles, F] partition-major."""
    f = arr2d.shape[1]
    return np.ascontiguousarray(
        arr2d.reshape(n_tiles, 128, f).transpose(1, 0, 2)
    )


def kernel(**inputs):
    x = np.asarray(inputs["x"], np.float32)
    W_in = np.asarray(inputs["W_in"], np.float32)
    b_in = np.asarray(inputs["b_in"], np.float32)
    w_edge = np.asarray(inputs["w_edge"], np.float32)
    W_out = np.asarray(inputs["W_out"], np.float32)
    b_out = np.asarray(inputs["b_out"], np.float32)
    edge_src = np.asarray(inputs["edge_src"]).astype(np.int64)
    edge_dst = np.asarray(inputs["edge_dst_local"]).astype(np.int64)
    offsets = np.asarray(inputs["edge_offsets"]).astype(np.int64)
    out_verts = np.asarray(inputs["out_verts"]).astype(np.int64)

    # ---- host-side packing ----
    # A: per-layer dense adjacency, padded 512/layer, bf16, chunk-major
    A = np.zeros((A_ROWS, PAD), np.float32)
    base = 0
    for l in range(1, L):
        s, e = int(offsets[l - 1]), int(offsets[l])
        rows = base + (edge_src[s:e] // PER) * PAD + (edge_src[s:e] % PER)
        np.add.at(A, (rows, edge_dst[s:e]), w_edge[s:e])
        base += l * PAD

    # Fold out_verts into layer 7: highway vertices (ids < (L-1)*PER)
    # pass through to pad dsts 500..500+n_hw-1; pad dst 511 carries a
    # constant 1.0 sourced from layer-0 pad vertex 511 (bias row).
    base7 = PAD * ((L - 1) * (L - 2) // 2)          # 10752
    hw_verts = out_verts[out_verts < (L - 1) * PER]
    n_hw = len(hw_verts)
    assert n_hw <= MAX_HW, n_hw
    assert np.array_equal(
        out_verts[n_hw:], np.arange((L - 1) * PER, L * PER)
    ), "out_verts tail must be the full last layer"
    for i, v in enumerate(hw_verts):
        p_v = (v // PER) * PAD + (v % PER)
        A[base7 + p_v, PER + i] = 1.0
    A[base7 + 511, 511] = 1.0                        # ones pass-through
    A_re = np.ascontiguousarray(
        A.reshape(N_CHUNK, 4, 128, PAD).transpose(0, 2, 1, 3)
    ).astype(bfloat16)

    winT = np.zeros((K_IN, PAD), np.float32)
    winT[:IN_DIM, :PER] = W_in.T
    # [896, 512] -> [128, 7 kt, 4 m, 128]
    winT_re = np.ascontiguousarray(
        _pack_ptiles(winT, 7).reshape(128, 7, 4, 128)
    ).astype(bfloat16)

    binP = np.zeros((PAD,), np.float32)
    binP[:PER] = b_in
    binP[511] = 1.0                                  # constant-1 pad vertex
    binP_re = np.ascontiguousarray(binP.reshape(4, 128).T)

    # Output head over layer-7 padded positions: q<500 -> last-layer
    # vertex, q=500+i -> highway i, q=511 -> bias row.
    woutP = np.zeros((PAD, OUT_DIM), np.float32)
    woutP[:PER, :] = W_out[:, n_hw:].T
    woutP[PER:PER + n_hw, :] = W_out[:, :n_hw].T
    woutP[511, :] = b_out
    woutP_re = np.ascontiguousarray(
        woutP.reshape(4, 128, OUT_DIM).transpose(1, 0, 2)
    ).astype(bfloat16)

    shared = {
        "W_inT": winT_re,
        "b_inP": binP_re,
        "A": A_re,
        "W_outP": woutP_re,
    }
    in_maps = []
    for c in range(NC):
        xT = np.zeros((K_IN, BL), np.float32)
        xT[:IN_DIM, :] = x[c * BL:(c + 1) * BL, :].T
        in_maps.append({"xT": _pack_ptiles(xT, 7).astype(bfloat16), **shared})

    from concourse.bass_utils import run_bass_kernel_spmd

    global _LAST_IN_MAPS, _PROG
    _LAST_IN_MAPS = in_maps
    if _PROG is None:
        _PROG = _build_program()
    res = run_bass_kernel_spmd(_PROG, in_maps, list(range(NC)))
    # out[c] is [128, 2, 10]: partition p, half hh -> batch hh*128+p
    out = np.concatenate(
        [
            np.asarray(res.results[c]["out"], np.float32)
            .transpose(1, 0, 2).reshape(BL, OUT_DIM)
            for c in range(NC)
        ],
        axis=0,
    )
    return np.ascontiguousarray(out)
